# revision 7
# baseline (speedup 1.0000x reference)
"""Trainium2 Bass kernel for nn_KernelLinear_60292750901529 (retrieval_knn).

Computes out[B, O] = log(exp(-sqrt(max(||x||^2 + ||w||^2 - 2 x.w, 0)) / 2))
                   = -0.5 * sqrt(max(d2, 0))
for x: [65536, 128] f32, w: [1024, 128] f32, sharded data-parallel over 8
NeuronCores (8192 rows each, weight replicated).

Per-core pipeline, per 128-row tile:
  DMA x tile -> DVE square+rowsum in f32 (0.25*x2 bias); DVE cast x to
  bf16 -> PE transpose (xT) -> PE bf16 GEMM into f32 PSUM: -2*x.wT, plus
  K=1 rank-1 update adding w2 ->
  ACT: u = Sqrt(0.25*psum + 0.25*x2)  (= 0.5*sqrt(d2), free affine+bias) ->
  GpSimd: y = -u -> DMA out (contiguous 512KB per tile).
"""

import numpy as np

BATCH = 65536
IN_F = 128
OUT_F = 1024
NCORES = 8
ROWS = BATCH // NCORES  # 8192 rows per core
RTILE = 128             # rows per tile (partition dim)
NTILES = ROWS // RTILE  # 64
NHALF = OUT_F // 512    # 2 matmuls of N=512 per tile

_compiled = {}


def _build(rows):
    import concourse.tile as tile
    from concourse import bacc, mybir

    ntiles = rows // RTILE
    f32 = mybir.dt.float32
    bf16 = mybir.dt.bfloat16

    nc = bacc.Bacc(
        "TRN2", target_bir_lowering=False, debug=False, num_devices=NCORES
    )
    x = nc.dram_tensor("x", [rows, IN_F], f32, kind="ExternalInput").ap()
    wTm2 = nc.dram_tensor("wTm2", [IN_F, OUT_F], bf16, kind="ExternalInput").ap()
    w2r = nc.dram_tensor("w2row", [1, OUT_F], bf16, kind="ExternalInput").ap()
    ones = nc.dram_tensor("ones", [1, RTILE], bf16, kind="ExternalInput").ap()
    ident = nc.dram_tensor("ident", [RTILE, RTILE], bf16, kind="ExternalInput").ap()
    out = nc.dram_tensor("out", [rows, OUT_F], f32, kind="ExternalOutput").ap()

    with tile.TileContext(nc) as tc:
        with (
            tc.tile_pool(name="consts", bufs=1) as cpool,
            tc.tile_pool(name="xin", bufs=4) as xpool,
            tc.tile_pool(name="xt", bufs=3) as xtpool,
            tc.tile_pool(name="sq", bufs=2) as sqpool,
            tc.tile_pool(name="bias", bufs=4) as bpool,
            tc.tile_pool(name="pt", bufs=2, space="PSUM") as ptpool,
            tc.tile_pool(name="pg", bufs=2, space="PSUM") as pgpool,
            tc.tile_pool(name="u", bufs=3) as upool,
            tc.tile_pool(name="y", bufs=3) as ypool,
        ):
            wTm2_s = cpool.tile([IN_F, OUT_F], bf16)
            nc.sync.dma_start(wTm2_s[:], wTm2[:])
            w2_s = cpool.tile([1, OUT_F], bf16)
            nc.sync.dma_start(w2_s[:], w2r[:])
            ones_s = cpool.tile([1, RTILE], bf16)
            nc.sync.dma_start(ones_s[:], ones[:])
            id_s = cpool.tile([RTILE, RTILE], bf16)
            nc.sync.dma_start(id_s[:], ident[:])

            for i in range(ntiles):
                xt_ = xpool.tile([RTILE, IN_F], f32, tag="x")
                nc.sync.dma_start(xt_[:], x[i * RTILE:(i + 1) * RTILE, :])

                # 0.25*||x_r||^2 per row (per-partition bias for the ACT).
                sq_ = sqpool.tile([RTILE, IN_F], f32, tag="sq")
                nc.vector.tensor_mul(sq_[:], xt_[:], xt_[:])
                b_ = bpool.tile([RTILE, 1], f32, tag="b")
                nc.vector.reduce_sum(b_[:], sq_[:], axis=mybir.AxisListType.X)
                b4_ = bpool.tile([RTILE, 1], f32, tag="b4")
                nc.vector.tensor_scalar_mul(b4_[:], b_[:], 0.25)

                # xT via PE transpose in bf16 (features onto partitions).
                xb_ = xpool.tile([RTILE, IN_F], bf16, tag="xb")
                nc.vector.tensor_copy(xb_[:], xt_[:])
                xTp = ptpool.tile([RTILE, RTILE], bf16, tag="xTp")
                nc.tensor.transpose(xTp[:], xb_[:], id_s[:])
                xTs = xtpool.tile([RTILE, RTILE], bf16, tag="xTs")
                nc.vector.tensor_copy(xTs[:], xTp[:])

                # PSUM g = -2*x.wT + w2 (rank-1 accumulate), fp32r rate.
                g_ = pgpool.tile([RTILE, OUT_F], f32, tag="g")
                for j in range(NHALF):
                    cs = slice(j * 512, (j + 1) * 512)
                    nc.tensor.matmul(
                        g_[:, cs],
                        xTs[:],
                        wTm2_s[:, cs],
                        start=True,
                        stop=False,
                    )
                    nc.tensor.matmul(
                        g_[:, cs],
                        ones_s[:],
                        w2_s[:, cs],
                        start=False,
                        stop=True,
                    )

                # u = sqrt(0.25*g + 0.25*x2) = 0.5*sqrt(d2)
                u_ = upool.tile([RTILE, OUT_F], f32, tag="u")
                nc.scalar.activation(
                    u_[:],
                    g_[:],
                    mybir.ActivationFunctionType.Sqrt,
                    bias=b4_[:],
                    scale=0.25,
                )
                # y = -u
                y_ = ypool.tile([RTILE, OUT_F], f32, tag="y")
                nc.gpsimd.tensor_scalar_mul(y_[:], u_[:], -1.0)
                nc.sync.dma_start(out[i * RTILE:(i + 1) * RTILE, :], y_[:])

    nc.compile()
    return nc


def get_nc(rows=ROWS):
    if rows not in _compiled:
        _compiled[rows] = _build(rows)
    return _compiled[rows]


def make_in_maps(input, weight, rows=ROWS):
    import ml_dtypes

    bf = ml_dtypes.bfloat16
    x = np.ascontiguousarray(input, dtype=np.float32)
    w = np.ascontiguousarray(weight, dtype=np.float32)
    wTm2 = np.ascontiguousarray((-2.0 * w.T).astype(bf))
    w2row = np.ascontiguousarray(
        (w * w).sum(axis=1, dtype=np.float32)[None, :].astype(bf)
    )
    ones = np.ones((1, RTILE), dtype=bf)
    ident = np.eye(RTILE, dtype=np.float32).astype(bf)
    n = x.shape[0] // rows
    return [
        {
            "x": x[c * rows:(c + 1) * rows],
            "wTm2": wTm2,
            "w2row": w2row,
            "ones": ones,
            "ident": ident,
        }
        for c in range(n)
    ]


def kernel(input, weight):
    from concourse.bass_utils import run_bass_kernel_spmd

    nc = get_nc()
    in_maps = make_in_maps(input, weight)
    res = run_bass_kernel_spmd(nc, in_maps, list(range(NCORES)))
    return np.concatenate([res.results[c]["out"] for c in range(NCORES)], axis=0)


# revision 8
# speedup vs baseline: 4.8928x; 4.8928x over previous
"""Trainium2 Bass kernel for nn_KernelLinear_60292750901529 (retrieval_knn).

Computes out[B, O] = log(exp(-sqrt(max(||x||^2 + ||w||^2 - 2 x.w, 0)) / 2))
                   = -0.5 * sqrt(max(d2, 0))
for x: [65536, 128] f32, w: [1024, 128] f32, sharded data-parallel over 8
NeuronCores (8192 rows each, weight replicated).

Per-core pipeline, per 128-row tile:
  DMA x tile -> DVE square+rowsum in f32 (0.25*x2 bias); DVE cast x to
  bf16 -> PE transpose (xT) -> PE bf16 GEMM into f32 PSUM: -2*x.wT, plus
  K=1 rank-1 update adding w2 ->
  ACT: u = Sqrt(0.25*psum + 0.25*x2)  (= 0.5*sqrt(d2), free affine+bias) ->
  GpSimd: y = -u -> DMA out (contiguous 512KB per tile).
"""

import numpy as np

BATCH = 65536
IN_F = 128
OUT_F = 1024
NCORES = 8
ROWS = BATCH // NCORES  # 8192 rows per core
RTILE = 128             # rows per tile (partition dim)
NTILES = ROWS // RTILE  # 64
NHALF = OUT_F // 512    # 2 matmuls of N=512 per tile

_compiled = {}


def _build(rows):
    import concourse.tile as tile
    from concourse import bacc, mybir

    ntiles = rows // RTILE
    f32 = mybir.dt.float32
    bf16 = mybir.dt.bfloat16

    nc = bacc.Bacc(
        "TRN2", target_bir_lowering=False, debug=False, num_devices=NCORES
    )
    x = nc.dram_tensor("x", [rows, IN_F], f32, kind="ExternalInput").ap()
    wTm2 = nc.dram_tensor("wTm2", [IN_F, OUT_F], bf16, kind="ExternalInput").ap()
    w2r = nc.dram_tensor("w2row", [1, OUT_F], bf16, kind="ExternalInput").ap()
    ones = nc.dram_tensor("ones", [1, RTILE], bf16, kind="ExternalInput").ap()
    ident = nc.dram_tensor("ident", [RTILE, RTILE], bf16, kind="ExternalInput").ap()
    out = nc.dram_tensor("out", [rows, OUT_F], f32, kind="ExternalOutput").ap()

    with tile.TileContext(nc) as tc:
        with (
            tc.tile_pool(name="consts", bufs=1) as cpool,
            tc.tile_pool(name="xin", bufs=4) as xpool,
            tc.tile_pool(name="xt", bufs=3) as xtpool,
            tc.tile_pool(name="sq", bufs=2) as sqpool,
            tc.tile_pool(name="bias", bufs=4) as bpool,
            tc.tile_pool(name="pt", bufs=2, space="PSUM") as ptpool,
            tc.tile_pool(name="pg", bufs=2, space="PSUM") as pgpool,
            tc.tile_pool(name="u", bufs=3) as upool,
            tc.tile_pool(name="y", bufs=3) as ypool,
        ):
            wTm2_s = cpool.tile([IN_F, OUT_F], bf16)
            nc.sync.dma_start(wTm2_s[:], wTm2[:])
            w2_s = cpool.tile([1, OUT_F], bf16)
            nc.sync.dma_start(w2_s[:], w2r[:])
            ones_s = cpool.tile([1, RTILE], bf16)
            nc.sync.dma_start(ones_s[:], ones[:])
            id_s = cpool.tile([RTILE, RTILE], bf16)
            nc.sync.dma_start(id_s[:], ident[:])

            for i in range(ntiles):
                xt_ = xpool.tile([RTILE, IN_F], f32, tag="x")
                nc.sync.dma_start(xt_[:], x[i * RTILE:(i + 1) * RTILE, :])

                # 0.25*||x_r||^2 per row (per-partition bias for the ACT).
                sq_ = sqpool.tile([RTILE, IN_F], f32, tag="sq")
                nc.vector.tensor_mul(sq_[:], xt_[:], xt_[:])
                b_ = bpool.tile([RTILE, 1], f32, tag="b")
                nc.vector.reduce_sum(b_[:], sq_[:], axis=mybir.AxisListType.X)
                b4_ = bpool.tile([RTILE, 1], f32, tag="b4")
                nc.vector.tensor_scalar_mul(b4_[:], b_[:], 0.25)

                # xT via PE transpose in bf16 (features onto partitions).
                xb_ = xpool.tile([RTILE, IN_F], bf16, tag="xb")
                nc.vector.tensor_copy(xb_[:], xt_[:])
                xTp = ptpool.tile([RTILE, RTILE], bf16, tag="xTp")
                nc.tensor.transpose(xTp[:], xb_[:], id_s[:])
                xTs = xtpool.tile([RTILE, RTILE], bf16, tag="xTs")
                nc.vector.tensor_copy(xTs[:], xTp[:])

                # PSUM g = -2*x.wT + w2 (rank-1 accumulate), fp32r rate.
                g_ = pgpool.tile([RTILE, OUT_F], f32, tag="g")
                for j in range(NHALF):
                    cs = slice(j * 512, (j + 1) * 512)
                    nc.tensor.matmul(
                        g_[:, cs],
                        xTs[:],
                        wTm2_s[:, cs],
                        start=True,
                        stop=False,
                    )
                    nc.tensor.matmul(
                        g_[:, cs],
                        ones_s[:],
                        w2_s[:, cs],
                        start=False,
                        stop=True,
                    )

                # u = sqrt(0.25*g + 0.25*x2) = 0.5*sqrt(d2)
                u_ = upool.tile([RTILE, OUT_F], f32, tag="u")
                nc.scalar.activation(
                    u_[:],
                    g_[:],
                    mybir.ActivationFunctionType.Sqrt,
                    bias=b4_[:],
                    scale=0.25,
                )
                # y = -u  (negate pass split 2:1 DVE:ACT to balance engines)
                y_ = ypool.tile([RTILE, OUT_F], f32, tag="y")
                if i % 3 == 2:
                    nc.scalar.mul(y_[:], u_[:], -1.0)
                else:
                    nc.vector.tensor_scalar_mul(y_[:], u_[:], -1.0)
                nc.sync.dma_start(out[i * RTILE:(i + 1) * RTILE, :], y_[:])

    nc.compile()
    return nc


def get_nc(rows=ROWS):
    if rows not in _compiled:
        _compiled[rows] = _build(rows)
    return _compiled[rows]


def make_in_maps(input, weight, rows=ROWS):
    import ml_dtypes

    bf = ml_dtypes.bfloat16
    x = np.ascontiguousarray(input, dtype=np.float32)
    w = np.ascontiguousarray(weight, dtype=np.float32)
    wTm2 = np.ascontiguousarray((-2.0 * w.T).astype(bf))
    w2row = np.ascontiguousarray(
        (w * w).sum(axis=1, dtype=np.float32)[None, :].astype(bf)
    )
    ones = np.ones((1, RTILE), dtype=bf)
    ident = np.eye(RTILE, dtype=np.float32).astype(bf)
    n = x.shape[0] // rows
    return [
        {
            "x": x[c * rows:(c + 1) * rows],
            "wTm2": wTm2,
            "w2row": w2row,
            "ones": ones,
            "ident": ident,
        }
        for c in range(n)
    ]


def kernel(input, weight):
    from concourse.bass_utils import run_bass_kernel_spmd

    nc = get_nc()
    in_maps = make_in_maps(input, weight)
    res = run_bass_kernel_spmd(nc, in_maps, list(range(NCORES)))
    return np.concatenate([res.results[c]["out"] for c in range(NCORES)], axis=0)
